# revision 41
# baseline (speedup 1.0000x reference)
"""NestedAttention Trainium2 kernel (v2).

Reference computation (per batch b):
  q_i = wq[i] @ x ; k_j = wk[j] @ x ; v_j = wv[j] @ x        (1x1 convs, r=64)
  for i: acc_i = sum_j softmax_m(q_i^T k_j / sqrt(r)) applied to v_j
  out = wo @ concat_i(acc_i) ; y = x * sigmoid(out)

Sharding: 8 cores = batch(4) x query-column-halves(2). Each core holds full
k/v (m = 2304 keys) and a 1152-wide slice of query columns n.

Changes over the bf16 baseline (237us -> ~218us):
  - mm2 (v @ E) runs in fp8e4 with MatmulPerfMode.DoubleRow (contraction over
    m in 9 k-tiles of 256 instead of 18 of 128) -> ~1.5x fewer PE cycles.
  - E is stored in fp8e4.  exp is split per m-tile between ScalarE
    (activation Exp -> fp8 out) and VectorE (Schraudolph bit-trick: since q is
    prescaled by log2(e) at projection time, the e4m3 BITS of exp(S/sqrt(r))
    are just trunc(logit + 56.5-C), computed by one tensor_scalar add+min
    into an int8 view of the E tile).  The min(...,119) clamp avoids TRN fp8
    Inf/NaN codes (>=120 = Inf/NaN since TRN e4m3 tops out at 240).
  - The softmax normalizer 1/Z comes from 64 replicated "ones" columns in the
    mm2 stationary; one 64-row partition-shifted copy + reciprocal gives the
    aligned reciprocals.  (reciprocal with a shifted source reads garbage on
    real HW even though CoreSim accepts it - only plain copies may shift.)
  - Accumulation adds (j>0) run on GPSIMD; sigmoid is computed as
    0.5*(1+tanh(z/2)) so exp+tanh share one ACT table set (host passes 0.5*x).
  - The final wo matmul + tanh + multiply + store are chunked into the last
    pair's normalization to shorten the tail.
  - Inputs are a single rotated x tensor per core (the core's n-slice first;
    valid because softmax/mm2 are permutation-invariant over keys m), serving
    q/k/v projections and the final x*sigmoid multiply (bf16 x there costs
    ~1e-3 extra L2, well inside the 2e-2 gate).

Notes from tuning on HW: row-tiled concurrent K=64 mm1 (tile_position (0,0)/
(64,0)) is ~1.8x faster in isolation but loses overall - two concurrent PSUM
drains starve the ScalarE/VectorE PSUM reads that the exp pipeline lives on.
"""

import os
import numpy as np

B, C, H, W = 4, 256, 48, 48
N = H * W            # 2304 keys (m) per image
NSLICE = N // 2      # 1152 query columns (n) per core
R = 64               # reduced channels
P = 128
MT = N // P          # 18 m-tiles
NG = MT // 2         # 9 double-row groups of 256 keys
KT = C // P          # 2 contraction tiles over channels
CHUNKS = [(0, 512), (512, 512), (1024, 128)]  # n chunks, PSUM-bank aligned
N_CORES = 8

LOG2E = float(np.log2(np.e))
ALPHA0 = LOG2E           # q prescale: logits arrive as 8*log2e*(S/sqrt(r))/8... see below
# We prescale q by 8*log2(e)/8 = log2(e) * (8/8)?  Derivation:
#   want bits = 8*(0.125*S*log2e) + 56.5 - C = (log2e*S) + 56.5 - C
#   so q is prescaled by log2e * 0.125 * 8 = log2e, i.e. mm1 emits Sb = log2e*S
#   ScalarE path: exp(0.125*S) = exp(Sb * ln2/8)   -> scale = ln(2)/8
ACT_SCALE = float(np.log(2.0) / 8.0)
BITS_C = float(os.environ.get("NESTED_BITS_C", "0.46"))
BITS_BIAS = 56.5 - BITS_C      # trunc() semantics of the int8 convert
BITS_CLAMP = 119.0             # max e4m3 bits (=240.0); >=120 is Inf/NaN on TRN

SSPLIT = int(os.environ.get("NESTED_SSPLIT", "11"))  # of 18 m-tiles on ScalarE
MID = int(os.environ.get("NESTED_MID", "7"))         # mm2/norm emission point
ADDS_ENGINE = os.environ.get("NESTED_ADDS", "gpsimd")    # gpsimd | dve
FINAL_ENGINE = os.environ.get("NESTED_FINAL", "dve")  # gpsimd | dve
# NOTE: the tiled-mm1 experiment additionally requires wq/wk column
# duplication in _host_prep (removed); do not enable without restoring it.
MM1_TILED = os.environ.get("NESTED_MM1_TILED", "0") == "1"
# recip source realignment: "shift" = recip reads rows 64:128 directly
# (broken on HW), "copy1" = one 64-row shifted copy, "copy2" = two 32-row
# shifted copies (baseline-proven)
RECIP_MODE = os.environ.get("NESTED_RECIP", "copy1")
if os.environ.get("NESTED_RECIP_SHIFT") == "0":
    RECIP_MODE = "copy2"

_CACHE = {}
LAST_RESULTS = None


def _dve_mts(ssplit):
    """Which m-tiles go to the VectorE Schraudolph path (spread evenly)."""
    d = MT - ssplit
    return set(mt for mt in range(MT) if (mt * d) % MT < d)


def _build_program():
    from contextlib import ExitStack

    import concourse.bass as bass
    import concourse.tile as tile
    from concourse import bacc, mybir

    f32 = mybir.dt.float32
    bf16 = mybir.dt.bfloat16
    fp8 = mybir.dt.float8e4
    i8 = mybir.dt.int8
    Exp = mybir.ActivationFunctionType.Exp
    Tanh = mybir.ActivationFunctionType.Tanh
    DoubleRow = mybir.MatmulPerfMode.DoubleRow
    mult = mybir.AluOpType.mult
    add = mybir.AluOpType.add
    min_op = mybir.AluOpType.min


    nc = bacc.Bacc("TRN2", target_bir_lowering=False, debug=False)
    # xb is column-rotated per core so the core's own n-slice is columns
    # 0:NSLICE (softmax/mm2 are permutation-invariant over keys m, so k/v/E
    # using the rotated order is fine).  One tensor feeds q, k, v and the
    # final x*sigmoid multiply.
    xb_d = nc.declare_dram_parameter("xb", [KT, P, N], bf16, isOutput=False)
    wqT_d = nc.declare_dram_parameter("wqT", [KT, P, 3, R], fp8, isOutput=False)
    wkT_d = nc.declare_dram_parameter("wkT", [KT, P, 3, R], fp8, isOutput=False)
    wvT_d = nc.declare_dram_parameter("wvT", [KT, P, 3, R], fp8, isOutput=False)
    x8_d = nc.declare_dram_parameter("x8", [KT, P, N], fp8, isOutput=False)
    woT_d = nc.declare_dram_parameter("woT", [3, R, C], bf16, isOutput=False)
    y_d = nc.declare_dram_parameter("y", [KT, P, NSLICE], f32, isOutput=True)

    with tile.TileContext(nc) as tc, ExitStack() as ctx:
        consts = ctx.enter_context(tc.tile_pool(name="consts", bufs=1))
        big_ps = ctx.enter_context(tc.tile_pool(name="big_ps", bufs=2, space="PSUM"))
        mm2_ps = ctx.enter_context(tc.tile_pool(name="mm2_ps", bufs=2, space="PSUM"))
        e_pool = ctx.enter_context(tc.tile_pool(name="e_pool", bufs=3))
        rb_pool = ctx.enter_context(tc.tile_pool(name="rb_pool", bufs=2))
        small = ctx.enter_context(tc.tile_pool(name="small", bufs=2))

        # ---- persistent SBUF state ----
        wqT_sb = consts.tile([P, KT, 3, R], fp8)
        nc.sync.dma_start(wqT_sb[:], wqT_d.rearrange("t p i r -> p t i r"))
        wkT_sb = consts.tile([P, KT, 3, R], fp8)
        nc.sync.dma_start(wkT_sb[:], wkT_d.rearrange("t p i r -> p t i r"))
        wvT_sb = consts.tile([P, KT, 3, R], fp8)
        nc.sync.dma_start(wvT_sb[:], wvT_d.rearrange("t p i r -> p t i r"))
        x8_sb = consts.tile([P, KT, N], fp8)
        nc.sync.dma_start(x8_sb[:, :, 0:NSLICE],
                          x8_d[:, :, 0:NSLICE].rearrange("t p m -> p t m"))
        nc.sync.dma_start(x8_sb[:, :, NSLICE:N],
                          x8_d[:, :, NSLICE:N].rearrange("t p m -> p t m"))
        x_sb = consts.tile([P, KT, N], bf16)
        nc.sync.dma_start(
            x_sb[:, :, 0:NSLICE], xb_d[:, :, 0:NSLICE].rearrange("t p m -> p t m")
        )
        nc.sync.dma_start(
            x_sb[:, :, NSLICE:N], xb_d[:, :, NSLICE:N].rearrange("t p m -> p t m")
        )

        woT_sb = []
        for i in range(3):
            w = consts.tile([P, C], bf16, tag=f"woT{i}")
            nc.gpsimd.memset(w[R:P, :], 0.0)
            nc.sync.dma_start(w[0:R, :], woT_d[i])
            woT_sb.append(w)

        q_sb = consts.tile([P, 3, NSLICE], bf16)
        nc.gpsimd.memset(q_sb[R:P, :, :], 0.0)
        k_sb = consts.tile([P, 3, N], bf16)
        nc.gpsimd.memset(k_sb[R:P, :, :], 0.0)

        # vT buffer per 256-group: [g, t, j, 0:64]=v_j fp8, [g, t, j, 64:128]=1
        vT_buf = consts.tile([P, NG, 2, 3, P], fp8)
        nc.gpsimd.memset(vT_buf[:, :, :, :, R:P], 1.0)

        # acc_i accumulated in bf16; rows 64-127 zero (K=128 pad for final mm)
        acc = []
        for i in range(3):
            a = consts.tile([P, NSLICE], bf16, tag=f"acc{i}")
            nc.gpsimd.memset(a[R:P, :], 0.0)
            acc.append(a)

        # warm the ACT exp table + GPSIMD tensor kernels during the DMA phase
        warm = consts.tile([P, 8], f32, tag="warm")
        nc.vector.memset(warm[:], 1.0)
        nc.scalar.activation(warm[:, 0:4], warm[:, 4:8], Exp, scale=0.1)
        nc.gpsimd.tensor_tensor(warm[:, 0:4], warm[:, 4:8], warm[:, 4:8], add)
        nc.gpsimd.tensor_scalar(warm[:, 0:4], warm[:, 4:8], 1.0, None, add)

        # ---- projections ----
        def emit_q(i):
            pt = big_ps.tile([P, NSLICE], f32, tag="big")
            for c0, cw in CHUNKS:
                nc.tensor.matmul(
                    pt[:R, c0 : c0 + cw],
                    wqT_sb[:, :, i, :],
                    x8_sb[:, :, c0 : c0 + cw],
                    start=True,
                    stop=True,
                    perf_mode=DoubleRow,
                )
            # prescale so mm1 logits come out as log2e * S
            nc.scalar.mul(q_sb[0:R, i, :], pt[0:R, :], ALPHA0)

        def emit_k(j, halves=(0, 1)):
            for half in halves:
                pt = big_ps.tile([P, NSLICE], f32, tag="big")
                for c0, cw in CHUNKS:
                    nc.tensor.matmul(
                        pt[:R, c0 : c0 + cw],
                        wkT_sb[:, :, j, :],
                        x8_sb[:, :, half * NSLICE + c0 : half * NSLICE + c0 + cw],
                        start=True,
                        stop=True,
                        perf_mode=DoubleRow,
                    )
                nc.scalar.copy(
                    k_sb[0:R, j, half * NSLICE : (half + 1) * NSLICE], pt[0:R, :]
                )

        def emit_vT_all():
            for mt in range(MT):
                g, t = mt // 2, mt % 2
                pv = mm2_ps.tile([P, 512], f32, tag="mm2")
                nc.tensor.matmul(
                    pv[:, 0 : 3 * R],
                    x8_sb[:, :, mt * P : (mt + 1) * P],
                    wvT_sb[:, :, :, :],
                    start=True,
                    stop=True,
                    perf_mode=DoubleRow,
                )
                nc.vector.tensor_copy(
                    vT_buf[:, g, t, :, 0:R],
                    pv[:, 0 : 3 * R].rearrange("p (j r) -> p j r", j=3),
                )

        # ---- attention pair pipeline ----
        def emit_exp(E, mt, pt, dve_mts):
            if mt in dve_mts:
                nc.vector.tensor_scalar(
                    E[:, mt, :].bitcast(i8),
                    pt[:],
                    BITS_BIAS,
                    BITS_CLAMP,
                    add,
                    min_op,
                )
            else:
                nc.scalar.activation(E[:, mt, :], pt[:], Exp, scale=ACT_SCALE)

        def emit_mm1_exp(i, j, E, mts, ssplit=SSPLIT):
            dve_mts = _dve_mts(ssplit)
            if MM1_TILED:
                # two concurrent 64-row PE tiles per m-tile pair (needs q/k
                # duplicated into partitions 64:128)
                steps = []
                mts = list(mts)
                while mts:
                    steps.append(tuple(mts[:2]))
                    mts = mts[2:]
                for step in steps:
                    pts = [
                        big_ps.tile([P, NSLICE], f32, tag="big", name=f"pt{s}")
                        for s in range(len(step))
                    ]
                    for c0, cw in CHUNKS:
                        for s, mt in enumerate(step):
                            half = 64 * s
                            nc.tensor.matmul(
                                pts[s][:, c0 : c0 + cw],
                                k_sb[half : half + R, j, mt * P : (mt + 1) * P],
                                q_sb[half : half + R, i, c0 : c0 + cw],
                                start=True,
                                stop=True,
                                tile_position=(half, 0),
                            )
                    for s, mt in enumerate(step):
                        emit_exp(E, mt, pts[s], _dve_mts(SSPLIT))
                return
            for mt in mts:
                pt = big_ps.tile([P, NSLICE], f32, tag="big")
                for c0, cw in CHUNKS:
                    nc.tensor.matmul(
                        pt[:, c0 : c0 + cw],
                        k_sb[:, j, mt * P : (mt + 1) * P],
                        q_sb[:, i, c0 : c0 + cw],
                        start=True,
                        stop=True,
                    )
                emit_exp(E, mt, pt, dve_mts)

        def emit_final_chunk(po, c0, cw):
            for ct in range(KT):
                for i in range(3):
                    nc.tensor.matmul(
                        po[ct][:, c0 : c0 + cw],
                        woT_sb[i][:, ct * P : (ct + 1) * P],
                        acc[i][:, c0 : c0 + cw],
                        start=(i == 0),
                        stop=(i == 2),
                    )
            for ct in range(KT):
                th = small.tile([P, 512], f32, tag=f"th{ct}")
                # sigmoid(z) = 0.5*(1+tanh(z/2)); xnh holds 0.5*x
                nc.scalar.activation(
                    th[:, 0:cw], po[ct][:, c0 : c0 + cw], Tanh, scale=0.5
                )
                y_sb = small.tile([P, 512], f32, tag=f"ysb{ct}")
                nc.vector.tensor_scalar(
                    th[:, 0:cw], th[:, 0:cw], 0.5, 0.5, mult, add
                )
                nc.vector.tensor_tensor(
                    y_sb[:, 0:cw],
                    x_sb[:, ct, c0 : c0 + cw],
                    th[:, 0:cw],
                    mult,
                )
                eng = nc.sync if ct == 0 else nc.scalar
                eng.dma_start(y_d[ct][:, c0 : c0 + cw], y_sb[:, 0:cw])

        def emit_mm2_norm(i, j, E, po=None, chunks=None):
            for c0, cw in chunks if chunks is not None else CHUNKS:
                pa = mm2_ps.tile([P, 512], f32, tag="mm2")
                for g in range(NG):
                    nc.tensor.matmul(
                        pa[:, 0:cw],
                        vT_buf[:, g, :, j, :],
                        E[:, 2 * g : 2 * g + 2, c0 : c0 + cw],
                        start=(g == 0),
                        stop=(g == NG - 1),
                        perf_mode=DoubleRow,
                    )
                rb = rb_pool.tile([R, 512], f32, tag="rb")
                if RECIP_MODE == "shift":
                    nc.vector.reciprocal_approx_fast(rb[:, 0:cw], pa[R:P, 0:cw])
                elif RECIP_MODE == "copy1":
                    nc.vector.tensor_copy(rb[:, 0:cw], pa[R:P, 0:cw])
                    nc.vector.reciprocal_approx_fast(rb[:, 0:cw], rb[:, 0:cw])
                else:
                    nc.vector.tensor_copy(rb[0:32, 0:cw], pa[R : R + 32, 0:cw])
                    nc.vector.tensor_copy(rb[32:R, 0:cw], pa[R + 32 : P, 0:cw])
                    nc.vector.reciprocal_approx_fast(rb[:, 0:cw], rb[:, 0:cw])
                if j == 0:
                    nc.vector.tensor_tensor(
                        acc[i][0:R, c0 : c0 + cw], pa[0:R, 0:cw], rb[:, 0:cw], mult
                    )
                else:
                    tmp = small.tile([R, 512], bf16, tag="tmp")
                    nc.vector.tensor_tensor(
                        tmp[:, 0:cw], pa[0:R, 0:cw], rb[:, 0:cw], mult
                    )
                    eng = nc.gpsimd if ADDS_ENGINE == "gpsimd" else nc.vector
                    eng.tensor_tensor(
                        acc[i][0:R, c0 : c0 + cw],
                        acc[i][0:R, c0 : c0 + cw],
                        tmp[:, 0:cw],
                        add,
                    )
                if po is not None:
                    emit_final_chunk(po, c0, cw)

        pairs = [(i, j) for j in range(3) for i in range(3)]
        cuts = [(MID, CHUNKS)]
        prev = None
        for idx, (i, j) in enumerate(pairs):
            E = e_pool.tile([P, MT, NSLICE], fp8, tag="E")
            if idx == 0:
                emit_q(0)
                emit_k(0, (0,))
            ssplit = SSPLIT
            lo = 0
            first_cut = True
            for cut, chks in cuts:
                emit_mm1_exp(i, j, E, range(lo, cut), ssplit)
                lo = cut
                if first_cut:
                    first_cut = False
                    if idx == 0:
                        emit_k(0, (1,))
                        emit_q(1)
                        emit_q(2)
                    elif idx == 1:
                        emit_vT_all()
                    elif idx == 2:
                        emit_k(1)
                    elif idx == 4:
                        emit_k(2)
                if prev is not None:
                    emit_mm2_norm(prev[0], prev[1], prev[2], chunks=chks)
            emit_mm1_exp(i, j, E, range(lo, MT), ssplit)
            prev = (i, j, E)
        po = [
            big_ps.tile([P, NSLICE], f32, tag="big", name="po0"),
            big_ps.tile([P, NSLICE], f32, tag="big", name="po1"),
        ]
        emit_mm2_norm(prev[0], prev[1], prev[2], po=po)

    nc.compile()
    return nc


def _get_program():
    if "nc" not in _CACHE:
        _CACHE["nc"] = _build_program()
    return _CACHE["nc"]


def _host_prep(x, wq, wk, wv, wo):
    import ml_dtypes

    bf16 = ml_dtypes.bfloat16
    xf = np.ascontiguousarray(x.reshape(B, C, N), dtype=np.float32)
    # wq: [3, R, C] -> wqT: [C, 3, R] -> [KT, P, 3, R]
    fp8 = ml_dtypes.float8_e4m3
    wqT = np.ascontiguousarray(np.transpose(wq, (2, 0, 1)).reshape(KT, P, 3, R)).astype(fp8)
    wkT = np.ascontiguousarray(np.transpose(wk, (2, 0, 1)).reshape(KT, P, 3, R)).astype(fp8)
    wvT = np.ascontiguousarray(np.transpose(wv, (2, 0, 1)).reshape(KT, P, 3, R)).astype(fp8)
    # wo: [C, 3R] -> woT[i] = wo[:, 64i:64(i+1)].T
    woT = np.ascontiguousarray(
        np.stack([wo[:, R * i : R * (i + 1)].T for i in range(3)])
    ).astype(bf16)
    in_maps = []
    for core in range(N_CORES):
        b, h = core // 2, core % 2
        xcore = xf[b].reshape(KT, P, N)
        # rotate columns so this core's n-slice comes first
        xrot = np.concatenate(
            [
                xcore[:, :, h * NSLICE : (h + 1) * NSLICE],
                xcore[:, :, (1 - h) * NSLICE : (2 - h) * NSLICE],
            ],
            axis=2,
        )
        in_maps.append(
            {
                "xb": np.ascontiguousarray(xrot).astype(bf16),
                "x8": np.ascontiguousarray(xrot).astype(fp8),
                "wqT": wqT,
                "wkT": wkT,
                "wvT": wvT,
                "woT": woT,
            }
        )
    return in_maps


def kernel(x, wq, wk, wv, wo):
    global LAST_RESULTS
    from concourse.bass_utils import run_bass_kernel_spmd

    x = np.asarray(x)
    nc = _get_program()
    in_maps = _host_prep(
        x, np.asarray(wq), np.asarray(wk), np.asarray(wv), np.asarray(wo)
    )
    res = run_bass_kernel_spmd(nc, in_maps, core_ids=list(range(N_CORES)))
    LAST_RESULTS = res
    out = np.empty((B, C, N), np.float32)
    for core in range(N_CORES):
        b, h = core // 2, core % 2
        out[b][:, h * NSLICE : (h + 1) * NSLICE] = res.results[core]["y"].reshape(
            C, NSLICE
        )
    return out.reshape(B, C, H, W).astype(x.dtype, copy=False)


# revision 42
# speedup vs baseline: 1.0082x; 1.0082x over previous
"""NestedAttention Trainium2 kernel (v2).

Reference computation (per batch b):
  q_i = wq[i] @ x ; k_j = wk[j] @ x ; v_j = wv[j] @ x        (1x1 convs, r=64)
  for i: acc_i = sum_j softmax_m(q_i^T k_j / sqrt(r)) applied to v_j
  out = wo @ concat_i(acc_i) ; y = x * sigmoid(out)

Sharding: 8 cores = batch(4) x query-column-halves(2). Each core holds full
k/v (m = 2304 keys) and a 1152-wide slice of query columns n.

Changes over the bf16 baseline (237us -> ~218us):
  - mm2 (v @ E) runs in fp8e4 with MatmulPerfMode.DoubleRow (contraction over
    m in 9 k-tiles of 256 instead of 18 of 128) -> ~1.5x fewer PE cycles.
  - E is stored in fp8e4.  exp is split per m-tile between ScalarE
    (activation Exp -> fp8 out) and VectorE (Schraudolph bit-trick: since q is
    prescaled by log2(e) at projection time, the e4m3 BITS of exp(S/sqrt(r))
    are just trunc(logit + 56.5-C), computed by one tensor_scalar add+min
    into an int8 view of the E tile).  The min(...,119) clamp avoids TRN fp8
    Inf/NaN codes (>=120 = Inf/NaN since TRN e4m3 tops out at 240).
  - The softmax normalizer 1/Z comes from 64 replicated "ones" columns in the
    mm2 stationary; one 64-row partition-shifted copy + reciprocal gives the
    aligned reciprocals.  (reciprocal with a shifted source reads garbage on
    real HW even though CoreSim accepts it - only plain copies may shift.)
  - Accumulation adds (j>0) run on GPSIMD; sigmoid is computed as
    0.5*(1+tanh(z/2)) so exp+tanh share one ACT table set (host passes 0.5*x).
  - The final wo matmul + tanh + multiply + store are chunked into the last
    pair's normalization to shorten the tail.
  - Inputs are a single rotated x tensor per core (the core's n-slice first;
    valid because softmax/mm2 are permutation-invariant over keys m), serving
    q/k/v projections and the final x*sigmoid multiply (bf16 x there costs
    ~1e-3 extra L2, well inside the 2e-2 gate).

Notes from tuning on HW: row-tiled concurrent K=64 mm1 (tile_position (0,0)/
(64,0)) is ~1.8x faster in isolation but loses overall - two concurrent PSUM
drains starve the ScalarE/VectorE PSUM reads that the exp pipeline lives on.
"""

import os
import numpy as np

B, C, H, W = 4, 256, 48, 48
N = H * W            # 2304 keys (m) per image
NSLICE = N // 2      # 1152 query columns (n) per core
R = 64               # reduced channels
P = 128
MT = N // P          # 18 m-tiles
NG = MT // 2         # 9 double-row groups of 256 keys
KT = C // P          # 2 contraction tiles over channels
CHUNKS = [(0, 512), (512, 512), (1024, 128)]  # n chunks, PSUM-bank aligned
N_CORES = 8

LOG2E = float(np.log2(np.e))
ALPHA0 = LOG2E           # q prescale: logits arrive as 8*log2e*(S/sqrt(r))/8... see below
# We prescale q by 8*log2(e)/8 = log2(e) * (8/8)?  Derivation:
#   want bits = 8*(0.125*S*log2e) + 56.5 - C = (log2e*S) + 56.5 - C
#   so q is prescaled by log2e * 0.125 * 8 = log2e, i.e. mm1 emits Sb = log2e*S
#   ScalarE path: exp(0.125*S) = exp(Sb * ln2/8)   -> scale = ln(2)/8
ACT_SCALE = float(np.log(2.0) / 8.0)
BITS_C = float(os.environ.get("NESTED_BITS_C", "0.46"))
BITS_BIAS = 56.5 - BITS_C      # trunc() semantics of the int8 convert
BITS_CLAMP = 119.0             # max e4m3 bits (=240.0); >=120 is Inf/NaN on TRN

SSPLIT = int(os.environ.get("NESTED_SSPLIT", "11"))  # of 18 m-tiles on ScalarE
MID = int(os.environ.get("NESTED_MID", "7"))         # mm2/norm emission point
ADDS_ENGINE = os.environ.get("NESTED_ADDS", "gpsimd")    # gpsimd | dve
FINAL_ENGINE = os.environ.get("NESTED_FINAL", "dve")  # gpsimd | dve
# NOTE: the tiled-mm1 experiment additionally requires wq/wk column
# duplication in _host_prep (removed); do not enable without restoring it.
MM1_TILED = os.environ.get("NESTED_MM1_TILED", "0") == "1"
# recip source realignment: "shift" = recip reads rows 64:128 directly
# (broken on HW), "copy1" = one 64-row shifted copy, "copy2" = two 32-row
# shifted copies (baseline-proven)
RECIP_MODE = os.environ.get("NESTED_RECIP", "copy1")
if os.environ.get("NESTED_RECIP_SHIFT") == "0":
    RECIP_MODE = "copy2"

_CACHE = {}
LAST_RESULTS = None


def _dve_mts(ssplit):
    """Which m-tiles go to the VectorE Schraudolph path (spread evenly)."""
    d = MT - ssplit
    return set(mt for mt in range(MT) if (mt * d) % MT < d)


def _build_program():
    from contextlib import ExitStack

    import concourse.bass as bass
    import concourse.tile as tile
    from concourse import bacc, mybir

    f32 = mybir.dt.float32
    bf16 = mybir.dt.bfloat16
    fp8 = mybir.dt.float8e4
    i8 = mybir.dt.int8
    Exp = mybir.ActivationFunctionType.Exp
    Tanh = mybir.ActivationFunctionType.Tanh
    DoubleRow = mybir.MatmulPerfMode.DoubleRow
    mult = mybir.AluOpType.mult
    add = mybir.AluOpType.add
    min_op = mybir.AluOpType.min


    nc = bacc.Bacc("TRN2", target_bir_lowering=False, debug=False)
    # xb is column-rotated per core so the core's own n-slice is columns
    # 0:NSLICE (softmax/mm2 are permutation-invariant over keys m, so k/v/E
    # using the rotated order is fine).  One tensor feeds q, k, v and the
    # final x*sigmoid multiply.
    xb_d = nc.declare_dram_parameter("xb", [KT, P, N], bf16, isOutput=False)
    wqT_d = nc.declare_dram_parameter("wqT", [KT, P, 3, R], fp8, isOutput=False)
    wkT_d = nc.declare_dram_parameter("wkT", [KT, P, 3, R], fp8, isOutput=False)
    wvT_d = nc.declare_dram_parameter("wvT", [KT, P, 3, R], fp8, isOutput=False)
    x8_d = nc.declare_dram_parameter("x8", [KT, P, N], fp8, isOutput=False)
    woT_d = nc.declare_dram_parameter("woT", [3, R, C], bf16, isOutput=False)
    y_d = nc.declare_dram_parameter("y", [KT, P, NSLICE], f32, isOutput=True)

    with tile.TileContext(nc) as tc, ExitStack() as ctx:
        consts = ctx.enter_context(tc.tile_pool(name="consts", bufs=1))
        big_ps = ctx.enter_context(tc.tile_pool(name="big_ps", bufs=2, space="PSUM"))
        mm2_ps = ctx.enter_context(tc.tile_pool(name="mm2_ps", bufs=2, space="PSUM"))
        e_pool = ctx.enter_context(tc.tile_pool(name="e_pool", bufs=2))
        rb_pool = ctx.enter_context(tc.tile_pool(name="rb_pool", bufs=2))
        small = ctx.enter_context(tc.tile_pool(name="small", bufs=2))

        # ---- persistent SBUF state ----
        wqT_sb = consts.tile([P, KT, 3, R], fp8)
        nc.sync.dma_start(wqT_sb[:], wqT_d.rearrange("t p i r -> p t i r"))
        wkT_sb = consts.tile([P, KT, 3, R], fp8)
        nc.sync.dma_start(wkT_sb[:], wkT_d.rearrange("t p i r -> p t i r"))
        wvT_sb = consts.tile([P, KT, 3, R], fp8)
        nc.sync.dma_start(wvT_sb[:], wvT_d.rearrange("t p i r -> p t i r"))
        x8_sb = consts.tile([P, KT, N], fp8)
        nc.sync.dma_start(x8_sb[:, :, 0:NSLICE],
                          x8_d[:, :, 0:NSLICE].rearrange("t p m -> p t m"))
        nc.sync.dma_start(x8_sb[:, :, NSLICE:N],
                          x8_d[:, :, NSLICE:N].rearrange("t p m -> p t m"))
        x_sb = consts.tile([P, KT, N], bf16)
        nc.sync.dma_start(
            x_sb[:, :, 0:NSLICE], xb_d[:, :, 0:NSLICE].rearrange("t p m -> p t m")
        )
        nc.sync.dma_start(
            x_sb[:, :, NSLICE:N], xb_d[:, :, NSLICE:N].rearrange("t p m -> p t m")
        )

        woT_sb = []
        for i in range(3):
            w = consts.tile([P, C], bf16, tag=f"woT{i}")
            nc.gpsimd.memset(w[R:P, :], 0.0)
            nc.sync.dma_start(w[0:R, :], woT_d[i])
            woT_sb.append(w)

        q_sb = consts.tile([P, 3, NSLICE], bf16)
        nc.gpsimd.memset(q_sb[R:P, :, :], 0.0)
        k_sb = consts.tile([P, 3, N], bf16)
        nc.gpsimd.memset(k_sb[R:P, :, :], 0.0)

        # vT buffer per 256-group: [g, t, j, 0:64]=v_j fp8, [g, t, j, 64:128]=1
        vT_buf = consts.tile([P, NG, 2, 3, P], fp8)
        nc.gpsimd.memset(vT_buf[:, :, :, :, R:P], 1.0)

        # acc_i accumulated in bf16; rows 64-127 zero (K=128 pad for final mm)
        acc = []
        for i in range(3):
            a = consts.tile([P, NSLICE], bf16, tag=f"acc{i}")
            nc.gpsimd.memset(a[R:P, :], 0.0)
            acc.append(a)

        # warm the ACT exp table + GPSIMD tensor kernels during the DMA phase
        warm = consts.tile([P, 8], f32, tag="warm")
        nc.vector.memset(warm[:], 1.0)
        nc.scalar.activation(warm[:, 0:4], warm[:, 4:8], Exp, scale=0.1)
        nc.gpsimd.tensor_tensor(warm[:, 0:4], warm[:, 4:8], warm[:, 4:8], add)
        nc.gpsimd.tensor_scalar(warm[:, 0:4], warm[:, 4:8], 1.0, None, add)

        # ---- projections ----
        def emit_q(i):
            pt = big_ps.tile([P, NSLICE], f32, tag="big")
            for c0, cw in CHUNKS:
                nc.tensor.matmul(
                    pt[:R, c0 : c0 + cw],
                    wqT_sb[:, :, i, :],
                    x8_sb[:, :, c0 : c0 + cw],
                    start=True,
                    stop=True,
                    perf_mode=DoubleRow,
                )
            # prescale so mm1 logits come out as log2e * S
            nc.scalar.mul(q_sb[0:R, i, :], pt[0:R, :], ALPHA0)

        def emit_k(j, halves=(0, 1)):
            for half in halves:
                pt = big_ps.tile([P, NSLICE], f32, tag="big")
                for c0, cw in CHUNKS:
                    nc.tensor.matmul(
                        pt[:R, c0 : c0 + cw],
                        wkT_sb[:, :, j, :],
                        x8_sb[:, :, half * NSLICE + c0 : half * NSLICE + c0 + cw],
                        start=True,
                        stop=True,
                        perf_mode=DoubleRow,
                    )
                nc.scalar.copy(
                    k_sb[0:R, j, half * NSLICE : (half + 1) * NSLICE], pt[0:R, :]
                )

        def emit_vT_all():
            for mt in range(MT):
                g, t = mt // 2, mt % 2
                pv = mm2_ps.tile([P, 512], f32, tag="mm2")
                nc.tensor.matmul(
                    pv[:, 0 : 3 * R],
                    x8_sb[:, :, mt * P : (mt + 1) * P],
                    wvT_sb[:, :, :, :],
                    start=True,
                    stop=True,
                    perf_mode=DoubleRow,
                )
                nc.vector.tensor_copy(
                    vT_buf[:, g, t, :, 0:R],
                    pv[:, 0 : 3 * R].rearrange("p (j r) -> p j r", j=3),
                )

        # ---- attention pair pipeline ----
        def emit_exp(E, mt, pt, dve_mts):
            if mt in dve_mts:
                nc.vector.tensor_scalar(
                    E[:, mt, :].bitcast(i8),
                    pt[:],
                    BITS_BIAS,
                    BITS_CLAMP,
                    add,
                    min_op,
                )
            else:
                nc.scalar.activation(E[:, mt, :], pt[:], Exp, scale=ACT_SCALE)

        def emit_mm1_exp(i, j, E, mts, ssplit=SSPLIT):
            dve_mts = _dve_mts(ssplit)
            if MM1_TILED:
                # two concurrent 64-row PE tiles per m-tile pair (needs q/k
                # duplicated into partitions 64:128)
                steps = []
                mts = list(mts)
                while mts:
                    steps.append(tuple(mts[:2]))
                    mts = mts[2:]
                for step in steps:
                    pts = [
                        big_ps.tile([P, NSLICE], f32, tag="big", name=f"pt{s}")
                        for s in range(len(step))
                    ]
                    for c0, cw in CHUNKS:
                        for s, mt in enumerate(step):
                            half = 64 * s
                            nc.tensor.matmul(
                                pts[s][:, c0 : c0 + cw],
                                k_sb[half : half + R, j, mt * P : (mt + 1) * P],
                                q_sb[half : half + R, i, c0 : c0 + cw],
                                start=True,
                                stop=True,
                                tile_position=(half, 0),
                            )
                    for s, mt in enumerate(step):
                        emit_exp(E, mt, pts[s], _dve_mts(SSPLIT))
                return
            for mt in mts:
                pt = big_ps.tile([P, NSLICE], f32, tag="big")
                for c0, cw in CHUNKS:
                    nc.tensor.matmul(
                        pt[:, c0 : c0 + cw],
                        k_sb[:, j, mt * P : (mt + 1) * P],
                        q_sb[:, i, c0 : c0 + cw],
                        start=True,
                        stop=True,
                    )
                emit_exp(E, mt, pt, dve_mts)

        def emit_final_chunk(po, c0, cw):
            for ct in range(KT):
                for i in range(3):
                    nc.tensor.matmul(
                        po[ct][:, c0 : c0 + cw],
                        woT_sb[i][:, ct * P : (ct + 1) * P],
                        acc[i][:, c0 : c0 + cw],
                        start=(i == 0),
                        stop=(i == 2),
                    )
            for ct in range(KT):
                th = small.tile([P, 512], f32, tag=f"th{ct}")
                # sigmoid(z) = 0.5*(1+tanh(z/2)); xnh holds 0.5*x
                nc.scalar.activation(
                    th[:, 0:cw], po[ct][:, c0 : c0 + cw], Tanh, scale=0.5
                )
                y_sb = small.tile([P, 512], f32, tag=f"ysb{ct}")
                nc.vector.tensor_scalar(
                    th[:, 0:cw], th[:, 0:cw], 0.5, 0.5, mult, add
                )
                nc.vector.tensor_tensor(
                    y_sb[:, 0:cw],
                    x_sb[:, ct, c0 : c0 + cw],
                    th[:, 0:cw],
                    mult,
                )
                eng = nc.sync if ct == 0 else nc.scalar
                eng.dma_start(y_d[ct][:, c0 : c0 + cw], y_sb[:, 0:cw])

        def emit_mm2_norm(i, j, E, po=None, chunks=None):
            for c0, cw in chunks if chunks is not None else CHUNKS:
                pa = mm2_ps.tile([P, 512], f32, tag="mm2")
                for g in range(NG):
                    nc.tensor.matmul(
                        pa[:, 0:cw],
                        vT_buf[:, g, :, j, :],
                        E[:, 2 * g : 2 * g + 2, c0 : c0 + cw],
                        start=(g == 0),
                        stop=(g == NG - 1),
                        perf_mode=DoubleRow,
                    )
                rb = rb_pool.tile([R, 512], f32, tag="rb")
                if RECIP_MODE == "shift":
                    nc.vector.reciprocal_approx_fast(rb[:, 0:cw], pa[R:P, 0:cw])
                elif RECIP_MODE == "copy1":
                    nc.vector.tensor_copy(rb[:, 0:cw], pa[R:P, 0:cw])
                    nc.vector.reciprocal_approx_fast(rb[:, 0:cw], rb[:, 0:cw])
                else:
                    nc.vector.tensor_copy(rb[0:32, 0:cw], pa[R : R + 32, 0:cw])
                    nc.vector.tensor_copy(rb[32:R, 0:cw], pa[R + 32 : P, 0:cw])
                    nc.vector.reciprocal_approx_fast(rb[:, 0:cw], rb[:, 0:cw])
                if j == 0:
                    nc.vector.tensor_tensor(
                        acc[i][0:R, c0 : c0 + cw], pa[0:R, 0:cw], rb[:, 0:cw], mult
                    )
                else:
                    tmp = small.tile([R, 512], bf16, tag="tmp")
                    nc.vector.tensor_tensor(
                        tmp[:, 0:cw], pa[0:R, 0:cw], rb[:, 0:cw], mult
                    )
                    eng = nc.gpsimd if ADDS_ENGINE == "gpsimd" else nc.vector
                    eng.tensor_tensor(
                        acc[i][0:R, c0 : c0 + cw],
                        acc[i][0:R, c0 : c0 + cw],
                        tmp[:, 0:cw],
                        add,
                    )
                if po is not None:
                    emit_final_chunk(po, c0, cw)

        pairs = [(i, j) for j in range(3) for i in range(3)]
        cuts = [(MID, CHUNKS)]
        prev = None
        for idx, (i, j) in enumerate(pairs):
            E = e_pool.tile([P, MT, NSLICE], fp8, tag="E")
            if idx == 0:
                emit_q(0)
                emit_k(0, (0,))
            ssplit = SSPLIT
            lo = 0
            first_cut = True
            for cut, chks in cuts:
                emit_mm1_exp(i, j, E, range(lo, cut), ssplit)
                lo = cut
                if first_cut:
                    first_cut = False
                    if idx == 0:
                        emit_k(0, (1,))
                        emit_q(1)
                        emit_q(2)
                    elif idx == 1:
                        emit_vT_all()
                    elif idx == 2:
                        emit_k(1)
                    elif idx == 4:
                        emit_k(2)
                if prev is not None:
                    emit_mm2_norm(prev[0], prev[1], prev[2], chunks=chks)
            emit_mm1_exp(i, j, E, range(lo, MT), ssplit)
            prev = (i, j, E)
        po = [
            big_ps.tile([P, NSLICE], f32, tag="big", name="po0"),
            big_ps.tile([P, NSLICE], f32, tag="big", name="po1"),
        ]
        emit_mm2_norm(prev[0], prev[1], prev[2], po=po)

    nc.compile()
    return nc


def _get_program():
    if "nc" not in _CACHE:
        _CACHE["nc"] = _build_program()
    return _CACHE["nc"]


def _host_prep(x, wq, wk, wv, wo):
    import ml_dtypes

    bf16 = ml_dtypes.bfloat16
    xf = np.ascontiguousarray(x.reshape(B, C, N), dtype=np.float32)
    # wq: [3, R, C] -> wqT: [C, 3, R] -> [KT, P, 3, R]
    fp8 = ml_dtypes.float8_e4m3
    wqT = np.ascontiguousarray(np.transpose(wq, (2, 0, 1)).reshape(KT, P, 3, R)).astype(fp8)
    wkT = np.ascontiguousarray(np.transpose(wk, (2, 0, 1)).reshape(KT, P, 3, R)).astype(fp8)
    wvT = np.ascontiguousarray(np.transpose(wv, (2, 0, 1)).reshape(KT, P, 3, R)).astype(fp8)
    # wo: [C, 3R] -> woT[i] = wo[:, 64i:64(i+1)].T
    woT = np.ascontiguousarray(
        np.stack([wo[:, R * i : R * (i + 1)].T for i in range(3)])
    ).astype(bf16)
    in_maps = []
    for core in range(N_CORES):
        b, h = core // 2, core % 2
        xcore = xf[b].reshape(KT, P, N)
        # rotate columns so this core's n-slice comes first
        xrot = np.concatenate(
            [
                xcore[:, :, h * NSLICE : (h + 1) * NSLICE],
                xcore[:, :, (1 - h) * NSLICE : (2 - h) * NSLICE],
            ],
            axis=2,
        )
        in_maps.append(
            {
                "xb": np.ascontiguousarray(xrot).astype(bf16),
                "x8": np.ascontiguousarray(xrot).astype(fp8),
                "wqT": wqT,
                "wkT": wkT,
                "wvT": wvT,
                "woT": woT,
            }
        )
    return in_maps


def kernel(x, wq, wk, wv, wo):
    global LAST_RESULTS
    from concourse.bass_utils import run_bass_kernel_spmd

    x = np.asarray(x)
    nc = _get_program()
    in_maps = _host_prep(
        x, np.asarray(wq), np.asarray(wk), np.asarray(wv), np.asarray(wo)
    )
    res = run_bass_kernel_spmd(nc, in_maps, core_ids=list(range(N_CORES)))
    LAST_RESULTS = res
    out = np.empty((B, C, N), np.float32)
    for core in range(N_CORES):
        b, h = core // 2, core % 2
        out[b][:, h * NSLICE : (h + 1) * NSLICE] = res.results[core]["y"].reshape(
            C, NSLICE
        )
    return out.reshape(B, C, H, W).astype(x.dtype, copy=False)
